# revision 23
# baseline (speedup 1.0000x reference)
"""DynamicMemoryRouter TRN2 Bass kernel (v2).

Sharding: 8 cores = B(4) x head-half(2). Core i handles batch b=i//2 and
head group g=i%2 (8 of 16 heads), then owns token half g after the og
exchange. All on-device activations are feature-major (X^T, features on
partitions).

Per-core pipeline:
  LN1: read X^T once ([128,4096] tiles), squares on the scalar engine,
  column sums via ones-matmuls (f32r moving), mean/rstd finalized in a
  packed [128,32] layout (DMA round trip), normalize the g-half in place.

  Attention (per head, no-max softmax -- scores are bounded ~|50| so
  exp is safe in f32): scores matmul -> exp (2-PSUM-bank chunks of 1024)
  with accumulated Z -> invZ via approx reciprocal -> fold invZ into the
  Mv stationary (plus a ones column producing the slot-renorm denom D)
  -> O accumulation -> D packed-reciprocal -> og = O*(1/(eps+D)) in bf16.

  og exchange: ReduceScatter-with-zeros. Each core writes BOTH g-row
  blocks of a [r, d-chunk, tokens] staging buffer -- its own og into one,
  zeros into the other -- selected by a host-provided per-core 0/1 mask
  folded into the slot-renorm multiply (addresses stay SPMD-uniform,
  only data diverges). The pairwise RS(add) then delivers
  og_full[d-chunk, own-token-half] to each rank at the same local
  address. Chunked 2x (per 4 heads) to overlap with remaining heads.
  Wo^T rows are host-permuted to match the chunk interleave.

  conv contracts full D for the own token half -> y = X^T + C kept
  SBUF-resident -> LN2 (packed stats) -> FFN with W1 SBUF-resident bf16,
  W2 streamed bf16, y/bias epilogue fused on the vector engine.

Matmuls: f32r / bf16 (1 row/cycle); fp32 PSUM accumulation.
"""

import os
import sys

for _p in ("/opt/trn_rl_repo", "/root/.axon_site/_ro/trn_rl_repo"):
    if os.path.isdir(_p) and _p not in sys.path:
        sys.path.insert(0, _p)

import numpy as np
import ml_dtypes

import concourse.bass as bass
import concourse.tile as tile
from concourse import bacc, mybir
from concourse.bass_utils import run_bass_kernel_spmd

F32 = mybir.dt.float32
F32R = mybir.dt.float32r
BF16 = mybir.dt.bfloat16
AF = mybir.ActivationFunctionType
ALU = mybir.AluOpType

B, N, D = 4, 4096, 1024
H, S = 16, 512
DH = D // H
DFF = 4 * D
P = 128
NC = 512
NH = 8            # local heads per core
NHALF = N // 2    # tokens owned post-exchange
LN_EPS = 1e-5
SLOT_EPS = 1e-9

_CACHED = {}


def _bcast_ap(dram_tile, row_offset_elems, width, parts):
    return bass.AP(
        tensor=dram_tile.tensor,
        offset=dram_tile.offset + row_offset_elems,
        ap=[[0, parts], [1, width]],
    )


def _emit_ln1(nc, tc, io, dr, cst, xg):
    """One pass over X^T: stats via ones-matmuls, packed finalize,
    in-place normalize of the g-half into xg."""
    with (
        tc.tile_pool(name="xoth", bufs=4) as xoth,
        tc.tile_pool(name="sqp", bufs=4) as sqp,
        tc.tile_pool(name="rows", bufs=1) as rows,
        tc.tile_pool(name="pkp", bufs=1) as pkp,
        tc.tile_pool(name="bcast", bufs=1) as bcast,
        tc.tile_pool(name="ps_st", bufs=4, space="PSUM") as ps_st,
    ):
        sum_row = rows.tile([1, N], F32, tag="sum_row", name="sum_row")
        sq_row = rows.tile([1, N], F32, tag="sq_row", name="sq_row")
        for nch in range(8):
            c0 = nch * NC
            ps_sum = ps_st.tile([1, NC], F32, tag="ps_st", name="ps_st")
            ps_sq = ps_st.tile([1, NC], F32, tag="ps_st", name="ps_st")
            for dt in range(8):
                if dt < 4:
                    ch = xg[dt][:, c0:c0 + NC]
                    nc.sync.dma_start(
                        out=ch, in_=io.xt[dt * P:(dt + 1) * P, c0:c0 + NC]
                    )
                else:
                    cht = xoth.tile([P, NC], F32R, tag="xo", name="xo")
                    nc.sync.dma_start(
                        out=cht, in_=io.xt[dt * P:(dt + 1) * P, c0:c0 + NC]
                    )
                    ch = cht[:, :]
                sq = sqp.tile([P, NC], F32R, tag="sq", name="sq")
                nc.scalar.square(sq[:, :], ch)
                nc.tensor.matmul(
                    ps_sum[:, :], cst.onesr[:, :], ch,
                    start=(dt == 0), stop=(dt == 7),
                )
                nc.tensor.matmul(
                    ps_sq[:, :], cst.onesr[:, :], sq[:, :],
                    start=(dt == 0), stop=(dt == 7),
                )
            nc.scalar.copy(sum_row[:, c0:c0 + NC], ps_sum[:, :])
            nc.scalar.copy(sq_row[:, c0:c0 + NC], ps_sq[:, :])

        nc.sync.dma_start(out=dr.s1d[0:1, :], in_=sum_row)
        nc.sync.dma_start(out=dr.s1d[1:2, :], in_=sq_row)

        # packed finalize: [1,4096] rows -> [128,32]
        pk = pkp.tile([P, 64], F32, tag="pk", name="pk")
        nc.sync.dma_start(
            out=pk[:, 0:32],
            in_=dr.s1d[0:1, :].rearrange("o (p c) -> (o p) c", p=P),
        )
        nc.sync.dma_start(
            out=pk[:, 32:64],
            in_=dr.s1d[1:2, :].rearrange("o (p c) -> (o p) c", p=P),
        )
        mean = pkp.tile([P, 32], F32, tag="mean", name="mean")
        nc.vector.tensor_scalar_mul(mean[:, :], pk[:, 0:32], 1.0 / D)
        msq = pkp.tile([P, 32], F32, tag="msq", name="msq")
        nc.vector.tensor_mul(msq[:, :], mean[:, :], mean[:, :])
        var = pkp.tile([P, 32], F32, tag="var", name="var")
        nc.vector.scalar_tensor_tensor(
            out=var[:, :], in0=pk[:, 32:64], scalar=1.0 / D,
            in1=msq[:, :], op0=ALU.mult, op1=ALU.subtract,
        )
        nc.vector.tensor_scalar_add(var[:, :], var[:, :], LN_EPS)
        sd = pkp.tile([P, 32], F32, tag="sd", name="sd")
        nc.scalar.sqrt(sd[:, :], var[:, :])
        rstd = pkp.tile([P, 32], F32, tag="rstd", name="rstd")
        nc.vector.reciprocal_approx_fast(out=rstd[:, :], in_=sd[:, :])
        nc.sync.dma_start(
            out=dr.r1d[0:1, :].rearrange("o (p c) -> (o p) c", p=P),
            in_=mean,
        )
        nc.sync.dma_start(
            out=dr.r1d[1:2, :].rearrange("o (p c) -> (o p) c", p=P),
            in_=rstd,
        )

        mb = bcast.tile([P, N], F32, tag="mb", name="mb")
        rb = bcast.tile([P, N], F32, tag="rb", name="rb")
        nc.sync.dma_start(out=mb, in_=_bcast_ap(dr.r1d, 0, N, P))
        nc.sync.dma_start(out=rb, in_=_bcast_ap(dr.r1d, N, N, P))

        for dt in range(4):
            nc.vector.tensor_sub(xg[dt][:, :], xg[dt][:, :], mb[:, :])
            nc.vector.tensor_mul(xg[dt][:, :], xg[dt][:, :], rb[:, :])
            nc.scalar.activation(
                out=xg[dt][:, :], in_=xg[dt][:, :], func=AF.Identity,
                bias=cst.lnbg_sb[:, dt:dt + 1], scale=cst.lngg_sb[:, dt:dt + 1],
            )


def _emit_attention(nc, tc, io, dr, cst, xg, groups, stage):
    """Per head: scores, no-max softmax over N, O accumulation with invZ
    folded into the stationary + slot renorm; og -> bf16 -> RS staging."""
    n_heads = int(os.environ.get("KERNEL_HEADS", str(NH)))
    with (
        tc.tile_pool(name="epool", bufs=8) as epool,
        tc.tile_pool(name="heads", bufs=2) as heads,
        tc.tile_pool(name="mvap", bufs=8) as mvap,
        tc.tile_pool(name="mvsp", bufs=8) as mvsp,
        tc.tile_pool(name="zp", bufs=2) as zp,
        tc.tile_pool(name="ogun", bufs=2) as ogun_pool,
        tc.tile_pool(name="ogbf", bufs=4) as ogbf_pool,
        tc.tile_pool(name="dbp", bufs=1) as dbp,
        tc.tile_pool(name="pkdp", bufs=4) as pkdp,
        tc.tile_pool(name="ps_sc", bufs=3, space="PSUM") as ps_sc,
        tc.tile_pool(name="ps_o", bufs=2, space="PSUM") as ps_o,
    ):
        for h in range(n_heads):
            hb = (h % 2) * 64
            mkt_h = heads.tile([P, S], F32R, tag="mkt_h", name="mkt_h")
            nc.sync.dma_start(out=mkt_h[hb:hb + 64, :], in_=io.mkt[h, :, :])
            mva = [mvap.tile([P, 65], F32, tag="mva", name="mva")
                   for _ in range(4)]
            for st in range(4):
                nc.sync.dma_start(
                    out=mva[st][:, 0:64], in_=io.mv[h, st * P:(st + 1) * P, :]
                )
                nc.sync.dma_start(out=mva[st][:, 64:65], in_=io.onesf[:, :])

            xg_h = xg[h // 2][hb:hb + 64, :]

            zc = zp.tile([P, 16], F32, tag="zc", name="zc")
            et = []
            for st in range(4):
                e_st = epool.tile([P, N], BF16, tag="e", name="e")
                for c2 in range(4):
                    ps = ps_sc.tile([P, 1024], F32, tag="ps_sc", name="ps_sc")
                    for k in range(2):
                        nc.tensor.matmul(
                            ps[:, k * NC:(k + 1) * NC],
                            mkt_h[hb:hb + 64, st * P:(st + 1) * P],
                            xg_h[:, c2 * 1024 + k * NC:c2 * 1024 + (k + 1) * NC],
                            start=True, stop=True,
                        )
                    nc.scalar.activation(
                        out=e_st[:, c2 * 1024:(c2 + 1) * 1024], in_=ps[:, :],
                        func=AF.Exp, accum_out=zc[:, st * 4 + c2:st * 4 + c2 + 1],
                    )
                if io.dbg16 is not None and h == 0:
                    nc.sync.dma_start(
                        out=io.dbg16[st * P:(st + 1) * P, :], in_=e_st
                    )
                et.append(e_st)

            Zt = zp.tile([P, 4], F32, tag="Zt", name="Zt")
            for st in range(4):
                nc.vector.reduce_sum(
                    out=Zt[:, st:st + 1], in_=zc[:, st * 4:(st + 1) * 4],
                    axis=mybir.AxisListType.X,
                )
            invZ = zp.tile([P, 4], F32, tag="invZ", name="invZ")
            nc.vector.reciprocal_approx_fast(out=invZ[:, :], in_=Zt[:, :])
            mvs = [mvsp.tile([P, 65], BF16, tag="mvs", name="mvs")
                   for _ in range(4)]
            for st in range(4):
                nc.vector.tensor_scalar_mul(
                    mvs[st][:, :], mva[st][:, :], invZ[:, st:st + 1]
                )

            og_un = ogun_pool.tile([65, N], BF16, tag="og_un", name="og_un")
            for nch in range(8):
                po = ps_o.tile([65, NC], F32, tag="ps_o", name="ps_o")
                for st in range(4):
                    nc.tensor.matmul(
                        po[:, :], mvs[st][:, :],
                        et[st][:, nch * NC:(nch + 1) * NC],
                        start=(st == 0), stop=(st == 3),
                    )
                nc.vector.tensor_copy(og_un[:, nch * NC:(nch + 1) * NC], po[:, :])

            if io.dbg16 is not None:
                nc.sync.dma_start(
                    out=io.dbg16[512 + h * 65:512 + (h + 1) * 65, :], in_=og_un
                )
            # slot-renorm denominator: packed reciprocal of (eps + D)
            nc.sync.dma_start(out=dr.dinv_raw[h:h + 1, :], in_=og_un[64:65, :])
            pkd = pkdp.tile([P, 32], BF16, tag="pkd", name="pkd")
            nc.sync.dma_start(
                out=pkd,
                in_=dr.dinv_raw[h:h + 1, :].rearrange("o (p c) -> (o p) c", p=P),
            )
            pkf = pkdp.tile([P, 32], F32, tag="pkf", name="pkf")
            nc.vector.tensor_scalar_add(pkf[:, :], pkd[:, :], SLOT_EPS)
            pki = pkdp.tile([P, 32], F32, tag="pki", name="pki")
            nc.vector.reciprocal_approx_fast(out=pki[:, :], in_=pkf[:, :])
            nc.sync.dma_start(
                out=dr.dinv_inv[h:h + 1, :].rearrange("o (p c) -> (o p) c", p=P),
                in_=pki,
            )
            db = dbp.tile([64, N], F32, tag="db", name="db")
            nc.sync.dma_start(out=db, in_=_bcast_ap(dr.dinv_inv, h * N, N, 64))
            # og * db, masked per g-block: own block gets og, peer block
            # zeros (gs0 = 1-g, gs1 = g), so the pairwise RS(add) yields
            # the concatenation with SPMD-uniform addressing.
            og_m = [ogbf_pool.tile([64, N], BF16, tag="og_m", name="og_m")
                    for _ in range(2)]
            for blk, gs in ((0, cst.gs0), (1, cst.gs1)):
                nc.vector.scalar_tensor_tensor(
                    out=og_m[blk][:, :], in0=og_un[0:64, :],
                    scalar=gs[0:64, 0:1], in1=db[:, :],
                    op0=ALU.mult, op1=ALU.mult,
                )
            hg = h // 4
            hr = (h % 4) * 64
            for r in range(2):
                for blk in range(2):
                    nc.sync.dma_start(
                        out=dr.rsin[hg][r, blk * 256 + hr:blk * 256 + hr + 64, :],
                        in_=og_m[blk][:, r * NHALF:(r + 1) * NHALF],
                    )
            if stage >= 3 and h % 4 == 3:
                nc.gpsimd.collective_compute(
                    "ReduceScatter", ALU.add, replica_groups=groups,
                    ins=[dr.rsin[hg][:, :, :]],
                    outs=[dr.rsout[hg * 512:(hg + 1) * 512, :]],
                )


def _emit_conv(nc, tc, io, dr, cst, y):
    """C = Wo^T @ og (full D contraction, own token half); y = X^T + C.
    LN2 stats (column sums of y and y^2) are folded in per token chunk."""
    with (
        tc.tile_pool(name="wotp", bufs=8) as wotp,
        tc.tile_pool(name="ogrd", bufs=8) as ogrd,
        tc.tile_pool(name="xthp", bufs=3) as xthp,
        tc.tile_pool(name="sq2p", bufs=2) as sq2p,
        tc.tile_pool(name="r2rows", bufs=4) as r2rows,
        tc.tile_pool(name="ps_c", bufs=4, space="PSUM") as ps_c,
        tc.tile_pool(name="ps_s2", bufs=4, space="PSUM") as ps_s2,
    ):
        io.wotr_sb = [wotp.tile([P, D], BF16, tag="wotr_sb", name="wotr_sb")
                      for _ in range(8)]
        for kc in range(8):
            nc.sync.dma_start(
                out=io.wotr_sb[kc], in_=io.wotr[kc * P:(kc + 1) * P, :]
            )
        for nch in range(4):
            mg = [ogrd.tile([P, NC], BF16, tag="mg", name="mg")
                  for _ in range(8)]
            for kc in range(8):
                nc.sync.dma_start(
                    out=mg[kc],
                    in_=dr.rsout[kc * P:(kc + 1) * P,
                                 nch * NC:(nch + 1) * NC],
                )
            for do in range(8):
                pc = ps_c.tile([P, NC], F32, tag="ps_c", name="ps_c")
                for kc in range(8):
                    nc.tensor.matmul(
                        pc[:, :], io.wotr_sb[kc][:, do * P:(do + 1) * P],
                        mg[kc][:, :], start=(kc == 0), stop=(kc == 7),
                    )
                xth_t = xthp.tile([P, NC], F32, tag="xth_t", name="xth_t")
                nc.sync.dma_start(
                    out=xth_t,
                    in_=io.xth[do * P:(do + 1) * P, nch * NC:(nch + 1) * NC],
                )
                nc.vector.tensor_add(
                    y[do][:, nch * NC:(nch + 1) * NC], pc[:, :], xth_t[:, :]
                )
            ps2_sum = ps_s2.tile([1, NC], F32, tag="ps2", name="ps2")
            ps2_sq = ps_s2.tile([1, NC], F32, tag="ps2", name="ps2")
            for dt in range(8):
                ysl = y[dt][:, nch * NC:(nch + 1) * NC]
                sq = sq2p.tile([P, NC], F32R, tag="sq2", name="sq2")
                nc.scalar.square(sq[:, :], ysl)
                nc.tensor.matmul(
                    ps2_sum[:, :], cst.onesr[:, :], ysl,
                    start=(dt == 0), stop=(dt == 7),
                )
                nc.tensor.matmul(
                    ps2_sq[:, :], cst.onesr[:, :], sq[:, :],
                    start=(dt == 0), stop=(dt == 7),
                )
            r2a = r2rows.tile([1, NC], F32, tag="r2row", name="r2row")
            r2b = r2rows.tile([1, NC], F32, tag="r2row", name="r2row")
            nc.scalar.copy(r2a[:, :], ps2_sum[:, :])
            nc.scalar.copy(r2b[:, :], ps2_sq[:, :])
            nc.sync.dma_start(
                out=dr.s2d[0:1, nch * NC:(nch + 1) * NC], in_=r2a
            )
            nc.sync.dma_start(
                out=dr.s2d[1:2, nch * NC:(nch + 1) * NC], in_=r2b
            )


def _emit_ln2(nc, tc, io, dr, cst, y, bc2):
    """LN2 packed finalize -> mb2/rb2 (stats accumulated during conv)."""
    with (
        tc.tile_pool(name="pk2p", bufs=1) as pk2p,
    ):

        pk = pk2p.tile([P, 32], F32, tag="pk2", name="pk2")
        nc.sync.dma_start(
            out=pk[:, 0:16],
            in_=dr.s2d[0:1, :].rearrange("o (p c) -> (o p) c", p=P),
        )
        nc.sync.dma_start(
            out=pk[:, 16:32],
            in_=dr.s2d[1:2, :].rearrange("o (p c) -> (o p) c", p=P),
        )
        mean = pk2p.tile([P, 16], F32, tag="mean2", name="mean2")
        nc.vector.tensor_scalar_mul(mean[:, :], pk[:, 0:16], 1.0 / D)
        msq = pk2p.tile([P, 16], F32, tag="msq2", name="msq2")
        nc.vector.tensor_mul(msq[:, :], mean[:, :], mean[:, :])
        var = pk2p.tile([P, 16], F32, tag="var2", name="var2")
        nc.vector.scalar_tensor_tensor(
            out=var[:, :], in0=pk[:, 16:32], scalar=1.0 / D,
            in1=msq[:, :], op0=ALU.mult, op1=ALU.subtract,
        )
        nc.vector.tensor_scalar_add(var[:, :], var[:, :], LN_EPS)
        sd = pk2p.tile([P, 16], F32, tag="sd2", name="sd2")
        nc.scalar.sqrt(sd[:, :], var[:, :])
        rstd = pk2p.tile([P, 16], F32, tag="rstd2", name="rstd2")
        nc.vector.reciprocal_approx_fast(out=rstd[:, :], in_=sd[:, :])
        nc.sync.dma_start(
            out=dr.r2d[0:1, :].rearrange("o (p c) -> (o p) c", p=P),
            in_=mean,
        )
        nc.sync.dma_start(
            out=dr.r2d[1:2, :].rearrange("o (p c) -> (o p) c", p=P),
            in_=rstd,
        )
        nc.sync.dma_start(out=bc2.mb2, in_=_bcast_ap(dr.r2d, 0, NHALF, P))
        nc.sync.dma_start(out=bc2.rb2, in_=_bcast_ap(dr.r2d, NHALF, NHALF, P))


def _emit_ffn(nc, tc, io, dr, cst, y, bc2):
    with (
        tc.tile_pool(name="h0p", bufs=8) as h0p,
        tc.tile_pool(name="h0tmp", bufs=2) as h0tmpp,
        tc.tile_pool(name="g1p", bufs=32) as g1p,
        tc.tile_pool(name="w2p", bufs=3) as w2p,
        tc.tile_pool(name="yo", bufs=4) as yop,
        tc.tile_pool(name="ps_f", bufs=8, space="PSUM") as ps_f,
    ):
        for tci in range(4):
            t0 = tci * NC
            h0c = [h0p.tile([P, NC], BF16, tag="h0c", name="h0c")
                   for _ in range(8)]
            for dt in range(8):
                ht = h0tmpp.tile([P, NC], F32, tag="h0tmp", name="h0tmp")
                nc.vector.tensor_sub(
                    ht[:, :], y[dt][:, t0:t0 + NC], bc2.mb2[:, t0:t0 + NC]
                )
                nc.vector.tensor_mul(ht[:, :], ht[:, :], bc2.rb2[:, t0:t0 + NC])
                nc.scalar.activation(
                    out=h0c[dt][:, :], in_=ht[:, :], func=AF.Identity,
                    bias=cst.ln2b_sb[:, dt:dt + 1], scale=cst.ln2g_sb[:, dt:dt + 1],
                )
            g1 = [g1p.tile([P, NC], BF16, tag="g1", name="g1") for _ in range(32)]
            for j in range(32):
                pm = ps_f.tile([P, NC], F32, tag="ps_f", name="ps_f")
                for kc in range(8):
                    nc.tensor.matmul(
                        pm[:, :], io.w1_sb[kc][:, j * P:(j + 1) * P],
                        h0c[kc][:, :], start=(kc == 0), stop=(kc == 7),
                    )
                nc.scalar.activation(
                    out=g1[j][:, :], in_=pm[:, :], func=AF.Gelu,
                    bias=cst.b1_sb[:, j:j + 1],
                )
            pms = [ps_f.tile([P, NC], F32, tag="ps_f", name="ps_f")
                   for _ in range(8)]
            for j in range(32):
                w2t = w2p.tile([P, D], BF16, tag="w2t", name="w2t")
                nc.sync.dma_start(out=w2t, in_=io.w2[j * P:(j + 1) * P, :])
                for k in range(8):
                    nc.tensor.matmul(
                        pms[k][:, :], w2t[:, k * P:(k + 1) * P],
                        g1[j][:, :], start=(j == 0), stop=(j == 31),
                    )
            for k in range(8):
                yo = yop.tile([P, NC], F32, tag="yo", name="yo")
                nc.vector.scalar_tensor_tensor(
                    out=yo[:, :], in0=pms[k][:, :],
                    scalar=cst.b2_sb[:, k:k + 1], in1=y[k][:, t0:t0 + NC],
                    op0=ALU.add, op1=ALU.add,
                )
                nc.sync.dma_start(
                    out=io.yout[k * P:(k + 1) * P, t0:t0 + NC], in_=yo
                )


class _NS:
    def __init__(self, **kw):
        self.__dict__.update(kw)


def build_nc(stage=6):
    nc = bacc.Bacc(None, target_bir_lowering=False, debug=False)

    io = _NS(
        xt=nc.dram_tensor("xt", [D, N], F32R, kind="ExternalInput"),
        xth=nc.dram_tensor("xth", [D, NHALF], F32, kind="ExternalInput"),
        mkt=nc.dram_tensor("mkt", [NH, DH, S], F32R, kind="ExternalInput"),
        mv=nc.dram_tensor("mv", [NH, S, DH], F32, kind="ExternalInput"),
        wotr=nc.dram_tensor("wotr", [D, D], BF16, kind="ExternalInput"),
        w1=nc.dram_tensor("w1", [D, DFF], BF16, kind="ExternalInput"),
        w2=nc.dram_tensor("w2", [DFF, D], BF16, kind="ExternalInput"),
        b1=nc.dram_tensor("b1", [DFF, 1], F32, kind="ExternalInput"),
        b2=nc.dram_tensor("b2", [D, 1], F32, kind="ExternalInput"),
        lngg=nc.dram_tensor("lngg", [D // 2, 1], F32, kind="ExternalInput"),
        lnbg=nc.dram_tensor("lnbg", [D // 2, 1], F32, kind="ExternalInput"),
        ln2g=nc.dram_tensor("ln2g", [D, 1], F32, kind="ExternalInput"),
        ln2b=nc.dram_tensor("ln2b", [D, 1], F32, kind="ExternalInput"),
        onesb=nc.dram_tensor("onesb", [P, 1], BF16, kind="ExternalInput"),
        onesf=nc.dram_tensor("onesf", [P, 1], F32, kind="ExternalInput"),
        gsel=nc.dram_tensor("gsel", [128, 1], F32, kind="ExternalInput"),
        yout=nc.dram_tensor("yout", [D, NHALF], F32, kind="ExternalOutput"),
    )
    debug = os.environ.get("KERNEL_DEBUG", "0") == "1"
    if debug:
        io.dbgf = nc.dram_tensor("dbgf", [1544, N], F32R, kind="ExternalOutput")
        io.dbg16 = nc.dram_tensor("dbg16", [2560, N], BF16, kind="ExternalOutput")
    else:
        io.dbgf = None
        io.dbg16 = None
    groups = [[0, 1], [2, 3], [4, 5], [6, 7]]

    with tile.TileContext(nc) as tc:
        with (
            tc.tile_pool(name="dram", bufs=1, space="DRAM") as dram,
            tc.tile_pool(name="consts", bufs=1) as consts,
        ):
            dr = _NS(
                s1d=dram.tile([2, N], F32, tag="s1d", name="s1d"),
                r1d=dram.tile([2, N], F32, tag="r1d", name="r1d"),
                s2d=dram.tile([2, NHALF], F32, tag="s2d", name="s2d"),
                r2d=dram.tile([2, NHALF], F32, tag="r2d", name="r2d"),
                dinv_raw=dram.tile([NH, N], BF16, tag="dinv_raw", name="dinv_raw"),
                dinv_inv=dram.tile([NH, N], F32, tag="dinv_inv", name="dinv_inv"),
                # RS staging: 2 chunks (4 heads each) of
                # [r(part), 2 g-blocks * 256 rows, NHALF]
                rsin=[dram.tile([2, 512, NHALF], BF16, tag=f"rsin{i}",
                                name=f"rsin{i}") for i in range(2)],
                rsout=dram.tile([D, NHALF], BF16, tag="rsout", name="rsout"),
            )

            def _load_col(name, src, cols):
                t = consts.tile([P, cols], F32, tag=name, name=name)
                nc.sync.dma_start(
                    out=t, in_=src[:, 0:1].rearrange("(j p) o -> p (j o)", p=P)
                )
                return t

            cst = _NS(
                ones16=consts.tile([P, 1], BF16, tag="ones16", name="ones16"),
                onesr=consts.tile([P, 1], F32R, tag="onesr", name="onesr"),
                b1_sb=_load_col("b1_sb", io.b1, DFF // P),
                b2_sb=_load_col("b2_sb", io.b2, D // P),
                lngg_sb=_load_col("lngg_sb", io.lngg, 4),
                lnbg_sb=_load_col("lnbg_sb", io.lnbg, 4),
                ln2g_sb=_load_col("ln2g_sb", io.ln2g, 8),
                ln2b_sb=_load_col("ln2b_sb", io.ln2b, 8),
            )
            nc.sync.dma_start(out=cst.ones16, in_=io.onesb[:, :])
            nc.sync.dma_start(out=cst.onesr, in_=io.onesf[:, :].bitcast(F32R))
            cst.gs1 = consts.tile([P, 1], F32, tag="gs1", name="gs1")
            nc.sync.dma_start(out=cst.gs1, in_=io.gsel[:, :])
            cst.gs0 = consts.tile([P, 1], F32, tag="gs0", name="gs0")
            nc.scalar.activation(
                out=cst.gs0, in_=cst.gs1, func=AF.Identity, bias=1.0, scale=-1.0
            )

            stagev = stage

            with tc.tile_pool(name="xg", bufs=4) as xg_pool:
                xg = [xg_pool.tile([P, N], F32R, tag="xg", name="xg")
                      for _ in range(4)]
                if stagev >= 1:
                    _emit_ln1(nc, tc, io, dr, cst, xg)
                    if debug:
                        for dt in range(4):
                            nc.sync.dma_start(
                                out=io.dbgf[dt * P:(dt + 1) * P, :], in_=xg[dt]
                            )
                        nc.sync.dma_start(
                            out=io.dbgf[1536:1538, :].bitcast(F32),
                            in_=dr.r1d[:, :],
                        )
                if stagev >= 2:
                    _emit_attention(nc, tc, io, dr, cst, xg, groups, stagev)

            with (
                tc.tile_pool(name="w1p", bufs=8) as w1p,
                tc.tile_pool(name="yp", bufs=8) as yp,
                tc.tile_pool(name="bc2p", bufs=1) as bc2p,
            ):
                io.w1_sb = [w1p.tile([P, DFF], BF16, tag="w1_sb", name="w1_sb")
                            for _ in range(8)]
                for kc in range(8):
                    nc.sync.dma_start(
                        out=io.w1_sb[kc], in_=io.w1[kc * P:(kc + 1) * P, :]
                    )
                y = [yp.tile([P, NHALF], F32R, tag="y", name="y")
                     for _ in range(8)]
                bc2 = _NS(
                    mb2=bc2p.tile([P, NHALF], F32, tag="mb2", name="mb2"),
                    rb2=bc2p.tile([P, NHALF], F32, tag="rb2", name="rb2"),
                )

                if stagev >= 4:
                    _emit_conv(nc, tc, io, dr, cst, y)
                    if debug:
                        nc.sync.dma_start(
                            out=io.dbg16[1032:2056, 0:NHALF], in_=dr.rsout[:, :]
                        )
                        for dt in range(8):
                            nc.sync.dma_start(
                                out=io.dbgf[512 + dt * P:512 + (dt + 1) * P, 0:NHALF],
                                in_=y[dt],
                            )
                if stagev >= 5:
                    _emit_ln2(nc, tc, io, dr, cst, y, bc2)
                if stagev >= 6:
                    _emit_ffn(nc, tc, io, dr, cst, y, bc2)

    nc.finalize()
    return nc


def _prep_inputs(F_in, Mk, Mv, ln_g, ln_b, Wo, ln2_g, ln2_b, W1, b1, W2, b2):
    f = np.asarray(F_in, np.float32)
    in_maps = []
    WoT = np.ascontiguousarray(np.asarray(Wo, np.float32).T)
    # permuted rows matching the RS chunk interleave (2 chunks of 4 heads):
    # d' = [hg: d hg*256:(hg+1)*256, d 512+hg*256:512+(hg+1)*256]
    perm = []
    for hg in range(2):
        perm.extend(range(hg * 256, (hg + 1) * 256))
        perm.extend(range(512 + hg * 256, 512 + (hg + 1) * 256))
    wotr = WoT[np.array(perm)].astype(ml_dtypes.bfloat16)
    W1c = np.ascontiguousarray(np.asarray(W1, np.float32)).astype(ml_dtypes.bfloat16)
    W2c = np.ascontiguousarray(np.asarray(W2, np.float32)).astype(ml_dtypes.bfloat16)
    b1c = np.ascontiguousarray(np.asarray(b1, np.float32).reshape(DFF, 1))
    b2c = np.ascontiguousarray(np.asarray(b2, np.float32).reshape(D, 1))
    ln2gc = np.ascontiguousarray(np.asarray(ln2_g, np.float32).reshape(D, 1))
    ln2bc = np.ascontiguousarray(np.asarray(ln2_b, np.float32).reshape(D, 1))
    onesb = np.ones((P, 1), ml_dtypes.bfloat16)
    onesf = np.ones((P, 1), np.float32)
    for core in range(8):
        b, g = core // 2, core % 2
        xtn = f[b].T                                           # (D, N)
        # own d-half first: the kernel normalizes rows 0:512 into xg
        xt = np.ascontiguousarray(
            np.concatenate([xtn[g * 512:(g + 1) * 512],
                            xtn[(1 - g) * 512:(2 - g) * 512]], axis=0))
        xth = np.ascontiguousarray(xtn[:, g * NHALF:(g + 1) * NHALF])
        hs = slice(g * NH, (g + 1) * NH)
        mkt = np.ascontiguousarray(
            np.asarray(Mk, np.float32)[hs].transpose(0, 2, 1))  # (8, DH, S)
        mv = np.ascontiguousarray(np.asarray(Mv, np.float32)[hs])
        lngg = np.ascontiguousarray(
            np.asarray(ln_g, np.float32)[g * 512:(g + 1) * 512].reshape(512, 1))
        lnbg = np.ascontiguousarray(
            np.asarray(ln_b, np.float32)[g * 512:(g + 1) * 512].reshape(512, 1))
        in_maps.append({
            "xt": xt, "xth": xth, "mkt": mkt, "mv": mv,
            "wotr": wotr, "w1": W1c, "w2": W2c, "b1": b1c, "b2": b2c,
            "lngg": lngg, "lnbg": lnbg, "ln2g": ln2gc, "ln2b": ln2bc,
            "onesb": onesb, "onesf": onesf,
            "gsel": np.full((128, 1), float(g), np.float32),
        })
    return in_maps


def run_on_hw(in_maps, **kwargs):
    stage = int(os.environ.get("KERNEL_STAGE", "6"))
    key = (stage, os.environ.get("KERNEL_HEADS"), os.environ.get("KERNEL_DEBUG"))
    if key not in _CACHED:
        _CACHED[key] = build_nc(stage)
    return run_bass_kernel_spmd(_CACHED[key], in_maps, list(range(8)), **kwargs)


def kernel(**inputs) -> np.ndarray:
    in_maps = _prep_inputs(**inputs)
    res = run_on_hw(in_maps)
    outs = [res.results[i]["yout"] for i in range(8)]
    full = np.empty((B, N, D), np.float32)
    for b in range(B):
        yt = np.concatenate([outs[2 * b], outs[2 * b + 1]], axis=1)  # (D, N)
        full[b] = yt.T
    return full
